# revision 1
# baseline (speedup 1.0000x reference)
"""Trainium2 Bass kernel for causal multi-head attention block (GPT-style).

Reference computation (fp32):
    qkv = x @ w_attn + b_attn          # [B,S,3E], heads interleaved per 192 cols
    q,k,v per head (d=64), scores = q k^T / 8, causal mask, softmax
    a = softmax @ v ; h = a @ w_proj + b_proj

Sharding (8 cores): core c -> batch b = c//4, head group g = c%4 (4 heads).
Each core computes qkv for its heads, full causal attention, and a partial
c_proj over its 256 e_in rows; a 4-way ReduceScatter(add) per batch group
yields each core's 512-token chunk of the final output. b_proj added on host.

Device layouts (host pre-marshals everything; fp32 has no DMA transpose):
    xT   [1024, 2048]   x[b]^T (e on partitions)
    wq   [128, 2, 8, 128]  per pair p: cols [qA|qB], PRE-SCALED by 1/8
    wk   [128, 2, 8, 128]  per pair p: cols [kA|kB]
    wv   [128, 8, 256]     4 heads' v cols side by side
    bq   [128, 2]  concat(bq_A,bq_B)/8 ; bk likewise unscaled
    bv   [128, 256]        v bias replicated across partitions
    tri  [128, 128]  upper-tri (key<=query) ; tri2 [128, 256] = [0 | tri]
    wp   [64, 4, 1024]     w_proj rows per head

On-device dataflow per head pair (heads stacked on partition halves):
    QT/KT [128, 2048] = w^T x^T via PE (fp32r), bias via ACT copy
    S^T[key,q] psum = KT_h^T QT_h (K=64, head A rows 0-63, head B 64-127)
    P = exp(S^T) via ACT (no max-sub needed: |scores| < ~3), tri-masked
    a^T|denom psum[65,512] += [V_h|1]^T P  (ones col gives softmax denom)
    at = a^T * recip(denom) broadcast  -> c_proj lhsT [64, tok]
"""

import os
import sys

import numpy as np

if "/opt/trn_rl_repo" not in sys.path:
    sys.path.insert(0, "/opt/trn_rl_repo")

B, S, E, H, D = 2, 2048, 1024, 16, 64
N_CORES = 8
PAIRS = 2  # head pairs per core
ET = 8  # e tiles of 128 over E=1024
QT_N = 4  # query tiles of 512
TT_N = 4  # token tiles of 512 (qkv QK rhs)
VT_N = 16  # token tiles of 128 (V / c_proj)

_cache = {}


def _build():
    import concourse.bass as bass
    import concourse.mybir as mybir
    import concourse.tile as tile
    from concourse import bacc
    from contextlib import ExitStack

    f32 = mybir.dt.float32
    f32r = mybir.dt.float32r
    bf16 = mybir.dt.bfloat16
    ALU = mybir.AluOpType
    AF = mybir.ActivationFunctionType

    nc = bacc.Bacc(
        "TRN2", target_bir_lowering=False, debug=False, num_devices=N_CORES
    )

    xT_d = nc.declare_dram_parameter("xT", [E, S], bf16, isOutput=False)
    wq_d = nc.declare_dram_parameter("wq", [128, PAIRS, ET, 128], bf16, isOutput=False)
    wk_d = nc.declare_dram_parameter("wk", [128, PAIRS, ET, 128], bf16, isOutput=False)
    wv_d = nc.declare_dram_parameter("wv", [128, ET, 256], bf16, isOutput=False)
    bq_d = nc.declare_dram_parameter("bq", [128, PAIRS], f32, isOutput=False)
    bk_d = nc.declare_dram_parameter("bk", [128, PAIRS], f32, isOutput=False)
    bv_d = nc.declare_dram_parameter("bv", [128, 256], f32, isOutput=False)
    tri_d = nc.declare_dram_parameter("tri", [128, 128], bf16, isOutput=False)
    wp_d = nc.declare_dram_parameter("wp", [128, PAIRS, 1024], bf16, isOutput=False)
    sel_d = nc.declare_dram_parameter("sel", [128, PAIRS, 128], bf16, isOutput=False)
    out_d = nc.declare_dram_parameter("out", [512, 1024], f32, isOutput=True)

    with ExitStack() as ctx:
        ctx.enter_context(
            nc.allow_low_precision(reason="fp32r tiles hold full fp32 bits in SBUF")
        )
        tc = ctx.enter_context(tile.TileContext(nc))
        const = ctx.enter_context(tc.tile_pool(name="const", bufs=1))
        dram = ctx.enter_context(tc.tile_pool(name="dram", bufs=1, space="DRAM"))
        psum = ctx.enter_context(tc.tile_pool(name="psum", bufs=4, space="PSUM"))
        psum_av = ctx.enter_context(tc.tile_pool(name="psum_av", bufs=2, space="PSUM"))
        pbuf = ctx.enter_context(tc.tile_pool(name="pbuf", bufs=6))

        # ---- persistent SBUF tensors ----
        xT = const.tile([128, ET, S], bf16, tag="xT")  # 8 MB
        wq = const.tile([128, PAIRS, ET, 128], bf16, tag="wq")
        wk = const.tile([128, PAIRS, ET, 128], bf16, tag="wk")
        wv = const.tile([128, ET, 256], bf16, tag="wv")
        bq = const.tile([128, PAIRS], f32, tag="bq")
        bk = const.tile([128, PAIRS], f32, tag="bk")
        bv = const.tile([128, 256], f32, tag="bv")
        tri = const.tile([128, 128], bf16, tag="tri")
        wp = const.tile([128, PAIRS, 1024], bf16, tag="wp")
        sel = const.tile([128, PAIRS, 128], bf16, tag="sel")
        den4 = const.tile([128, 512], f32, tag="den4")
        nc.vector.memset(den4[:], 1.0)
        qt_sb = const.tile([128, PAIRS, S], bf16, tag="qt")  # rows 0-63 head A
        kt_sb = const.tile([128, PAIRS, S], bf16, tag="kt")
        vv = const.tile([128, VT_N, 4 * 65], bf16, tag="vv")  # [key,tt,(h,d|1)]
        at = const.tile([128, PAIRS, S], bf16, tag="at")  # pair-stacked a^T

        # ---- input DMAs ----
        nc.sync.dma_start(out=wv[:], in_=wv_d[:])
        nc.sync.dma_start(out=wq[:], in_=wq_d[:])
        nc.sync.dma_start(out=wk[:], in_=wk_d[:])
        nc.sync.dma_start(out=bq[:], in_=bq_d[:])
        nc.sync.dma_start(out=bk[:], in_=bk_d[:])
        nc.sync.dma_start(out=bv[:], in_=bv_d[:])
        nc.sync.dma_start(out=tri[:], in_=tri_d[:])
        nc.sync.dma_start(out=wp[:], in_=wp_d[:])
        nc.sync.dma_start(out=sel[:], in_=sel_d[:])
        for et in range(ET):
            for hf in range(2):
                nc.sync.dma_start(
                    out=xT[:, et, hf * 1024 : (hf + 1) * 1024],
                    in_=xT_d[et * 128 : (et + 1) * 128, hf * 1024 : (hf + 1) * 1024],
                )
        nc.vector.memset(vv.rearrange("p t (h e) -> p t h e", h=4)[:, :, :, 64:65], 1.0)

        # ---- Phase A: QKV projections (V first: AV needs it earliest) ----
        for tt in range(VT_N):
            sl = slice(tt * 128, (tt + 1) * 128)
            ps_v = psum.tile([128, 256], f32, tag="mm")
            for et in range(ET):
                nc.tensor.matmul(
                    ps_v,
                    lhsT=xT[:, et, sl],
                    rhs=wv[:, et],
                    start=(et == 0),
                    stop=(et == ET - 1),
                )
            nc.vector.tensor_tensor(
                out=vv.rearrange("p t (h e) -> p t h e", h=4)[:, tt, :, 0:64],
                in0=ps_v.rearrange("p (h e) -> p h e", h=4),
                in1=bv.rearrange("p (h e) -> p h e", h=4),
                op=ALU.add,
            )
        for p in range(PAIRS):
            for tt in range(TT_N):
                sl = slice(tt * 512, (tt + 1) * 512)
                ps_q = psum.tile([128, 512], f32, tag="mm")
                for et in range(ET):
                    nc.tensor.matmul(
                        ps_q,
                        lhsT=wq[:, p, et],
                        rhs=xT[:, et, sl],
                        start=(et == 0),
                        stop=(et == ET - 1),
                    )
                nc.vector.tensor_scalar_add(qt_sb[:, p, sl], ps_q, bq[:, p : p + 1])
                ps_k = psum.tile([128, 512], f32, tag="mm")
                for et in range(ET):
                    nc.tensor.matmul(
                        ps_k,
                        lhsT=wk[:, p, et],
                        rhs=xT[:, et, sl],
                        start=(et == 0),
                        stop=(et == ET - 1),
                    )
                nc.vector.tensor_scalar_add(kt_sb[:, p, sl], ps_k, bk[:, p : p + 1])

        # ---- Phase B+C fused: per query-tile attention -> c_proj -> RS ----
        # c0 for the 4 diagonal key-tiles (j=3 widened to 256 so fp32r stays
        # at full rate; the extra cols are masked by tri2)
        diag_c0 = (0, 128, 256, 384)
        cc_in = []
        cc_out = []
        for qt in range(QT_N):
            cc_in.append(
                dram.tile([512, 1024], f32, tag=f"cc_in{qt}", name=f"cc_in{qt}")
            )
            cc_out.append(
                dram.tile([128, 1024], f32, tag=f"cc_out{qt}", name=f"cc_out{qt}")
            )
        cc_out_l = []
        for half in range(2):
            cc_out_l.append(
                dram.tile([64, 1024], f32, tag=f"cc_outl{half}", name=f"cc_outl{half}")
            )
        def flush(qt, den4, atu):
            """normalize (recip->broadcast->mult), c_proj, ReduceScatter for qt.

            Called one qt later so the reciprocal latency hides behind the
            next query-tile's score/AV stream on the PE."""
            rec4 = pbuf.tile(
                [128, 512], bf16, tag="recb", bufs=2, name=f"rec_{qt}"
            )
            nc.vector.reciprocal(rec4[:], den4[:])
            for pi in range(PAIRS):
                rb = psum.tile([128, 512], f32, tag="cc", bufs=2, name=f"rb_{qt}_{pi}")
                nc.tensor.matmul(
                    rb, lhsT=sel[:, pi, :], rhs=rec4[:], start=True, stop=True
                )
                nc.vector.tensor_tensor(
                    out=at[:, pi, qt * 512 : (qt + 1) * 512],
                    in0=atu[pi][:],
                    in1=rb[:],
                    op=ALU.mult,
                )
            for tt in range(4 * qt, 4 * qt + 4):
                for nt in range(2):
                    ps_c = psum.tile([128, 512], f32, tag="cc", bufs=2)
                    for pi in range(PAIRS):
                        nc.tensor.matmul(
                            ps_c,
                            lhsT=at[:, pi, tt * 128 : (tt + 1) * 128],
                            rhs=wp[:, pi, nt * 512 : (nt + 1) * 512],
                            start=(pi == 0),
                            stop=(pi == PAIRS - 1),
                        )
                    cst = pbuf.tile([128, 512], f32, tag="cstage", bufs=2, name=f"cst_{tt}_{nt}")
                    nc.scalar.copy(cst[:], ps_c[:])
                    nc.sync.dma_start(
                        out=cc_in[qt][
                            (tt - 4 * qt) * 128 : (tt - 4 * qt + 1) * 128,
                            nt * 512 : (nt + 1) * 512,
                        ],
                        in_=cst[:],
                    )
                # split the LAST query tile's ReduceScatter in two to shorten
                # the serial tail
                if qt == QT_N - 1 and tt in (4 * qt + 1, 4 * qt + 3):
                    half = 0 if tt == 4 * qt + 1 else 1
                    nc.gpsimd.collective_compute(
                        "ReduceScatter",
                        mybir.AluOpType.add,
                        replica_groups=[[0, 1, 2, 3], [4, 5, 6, 7]],
                        ins=[cc_in[qt][half * 256 : (half + 1) * 256, :].opt()],
                        outs=[cc_out_l[half][:].opt()],
                    )
                    nc.sync.dma_start(
                        out=out_d[qt * 128 + half * 64 : qt * 128 + (half + 1) * 64, :],
                        in_=cc_out_l[half][:],
                    )
            if qt < QT_N - 1:
                nc.gpsimd.collective_compute(
                    "ReduceScatter",
                    mybir.AluOpType.add,
                    replica_groups=[[0, 1, 2, 3], [4, 5, 6, 7]],
                    ins=[cc_in[qt][:].opt()],
                    outs=[cc_out[qt][:].opt()],
                )
                nc.sync.dma_start(
                    out=out_d[qt * 128 : (qt + 1) * 128, :], in_=cc_out[qt][:]
                )

        pending = None
        for qt in range(QT_N):
            if pending is not None:
                flush(*pending)
                pending = None
            den4 = pbuf.tile([128, 512], f32, tag="den", bufs=2, name=f"den_{qt}")
            nc.vector.memset(den4[:], 1.0)
            atu_pair = []
            for p in range(PAIRS):
                av = []
                for hh in range(2):
                    av.append(
                        psum_av.tile([65, 512], f32, tag="av", name=f"av_{p}_{qt}_{hh}")
                    )
                nkt = 4 * qt + 4
                for kt in range(nkt):
                    j = kt - 4 * qt
                    c0 = diag_c0[j] if j >= 0 else 0
                    n = 512 - c0
                    ps_s = []
                    pts = []
                    for hh in range(2):
                        base = hh * 64
                        ps_s.append(psum.tile([128, 512], f32, tag="mm",
                                              name=f"ps_s_{p}_{qt}_{kt}_{hh}"))
                        nc.tensor.matmul(
                            ps_s[hh][:, 0:n],
                            lhsT=kt_sb[base : base + 64, p, kt * 128 : (kt + 1) * 128],
                            rhs=qt_sb[base : base + 64, p, qt * 512 + c0 : (qt + 1) * 512],
                            start=True,
                            stop=True,
                        )
                    for hh in range(2):
                        pt = pbuf.tile([128, 512], bf16, tag="p", bufs=8,
                                       name=f"pt_{p}_{qt}_{kt}_{hh}")
                        pts.append(pt)
                        nc.scalar.activation(pt[:, 0:n], ps_s[hh][:, 0:n], AF.Exp)
                        if j >= 0:
                            nc.vector.tensor_tensor(
                                out=pt[:, 0:128], in0=pt[:, 0:128], in1=tri[:],
                                op=ALU.mult,
                            )
                    for hh in range(2):
                        h_idx = 2 * p + hh
                        nc.tensor.matmul(
                            av[hh][:, c0:512],
                            lhsT=vv[:, kt, h_idx * 65 : (h_idx + 1) * 65],
                            rhs=pts[hh][:, 0:n],
                            start=(kt == 0),
                            stop=(kt == nkt - 1),
                        )
                for hh in range(2):
                    h_idx = 2 * p + hh
                    nc.vector.tensor_copy(
                        out=den4[h_idx * 32 : h_idx * 32 + 1, :], in_=av[hh][64:65, :]
                    )
                atu2 = pbuf.tile([128, 512], f32, tag="atu", bufs=4,
                                 name=f"atu_{p}_{qt}")
                nc.vector.tensor_copy(out=atu2[0:64, :], in_=av[0][0:64, :])
                nc.vector.tensor_copy(out=atu2[64:128, :], in_=av[1][0:64, :])
                atu_pair.append(atu2)
            pending = (qt, den4, atu_pair)
        flush(*pending)

    nc.compile()
    return nc


def _prepare_in_maps(x, w_attn, b_attn, w_proj):
    import ml_dtypes

    bf = ml_dtypes.bfloat16
    in_maps = []
    tri = np.triu(np.ones((128, 128), dtype=bf))
    for core in range(N_CORES):
        b, g = core // 4, core % 4
        heads = [4 * g + i for i in range(4)]
        xT = np.ascontiguousarray(x[b].T)  # [1024, 2048]
        wq_blocks, wk_blocks, bq_cols, bk_cols = [], [], [], []
        for pr in range(PAIRS):
            hA, hB = heads[2 * pr], heads[2 * pr + 1]
            wq_blk = np.concatenate(
                [w_attn[:, hA * 192 : hA * 192 + 64], w_attn[:, hB * 192 : hB * 192 + 64]],
                axis=1,
            ) * 0.125
            wk_blk = np.concatenate(
                [
                    w_attn[:, hA * 192 + 64 : hA * 192 + 128],
                    w_attn[:, hB * 192 + 64 : hB * 192 + 128],
                ],
                axis=1,
            )
            # [1024,128] -> [128part, 8et, 128]
            wq_blocks.append(wq_blk.reshape(ET, 128, 128).transpose(1, 0, 2))
            wk_blocks.append(wk_blk.reshape(ET, 128, 128).transpose(1, 0, 2))
            bq_cols.append(
                np.concatenate(
                    [b_attn[hA * 192 : hA * 192 + 64], b_attn[hB * 192 : hB * 192 + 64]]
                ) * 0.125
            )
            bk_cols.append(
                np.concatenate(
                    [
                        b_attn[hA * 192 + 64 : hA * 192 + 128],
                        b_attn[hB * 192 + 64 : hB * 192 + 128],
                    ]
                )
            )
        wq_h = np.stack(wq_blocks, axis=1)  # [128, 2, 8, 128]
        wk_h = np.stack(wk_blocks, axis=1)
        wv_blk = np.concatenate(
            [w_attn[:, h * 192 + 128 : h * 192 + 192] for h in heads], axis=1
        )  # [1024, 256]
        wv_h = wv_blk.reshape(ET, 128, 256).transpose(1, 0, 2)  # [128, 8, 256]
        bv_row = np.concatenate(
            [b_attn[h * 192 + 128 : h * 192 + 192] for h in heads]
        )  # [256]
        bv_h = np.broadcast_to(bv_row, (128, 256)).copy()
        wp_h = np.empty((128, PAIRS, 1024), dtype=np.float32)
        sel_h = np.zeros((128, PAIRS, 128), dtype=np.float32)
        for pr in range(PAIRS):
            hA, hB = heads[2 * pr], heads[2 * pr + 1]
            wp_h[0:64, pr, :] = w_proj[hA * 64 : (hA + 1) * 64, :]
            wp_h[64:128, pr, :] = w_proj[hB * 64 : (hB + 1) * 64, :]
            sel_h[(2 * pr) * 32, pr, 0:64] = 1.0
            sel_h[(2 * pr + 1) * 32, pr, 64:128] = 1.0
        in_maps.append(
            {
                "xT": np.ascontiguousarray(xT.astype(bf)),
                "wq": np.ascontiguousarray(wq_h.astype(bf)),
                "wk": np.ascontiguousarray(wk_h.astype(bf)),
                "wv": np.ascontiguousarray(wv_h.astype(bf)),
                "bq": np.ascontiguousarray(np.stack(bq_cols, 1), dtype=np.float32),
                "bk": np.ascontiguousarray(np.stack(bk_cols, 1), dtype=np.float32),
                "bv": bv_h.astype(np.float32),
                "tri": tri,
                "wp": np.ascontiguousarray(wp_h.astype(bf)),
                "sel": np.ascontiguousarray(sel_h.astype(bf)),
            }
        )
    return in_maps


def _run(x, w_attn, b_attn, w_proj, b_proj, trace=False):
    from concourse.bass_utils import run_bass_kernel_spmd

    if "nc" not in _cache:
        _cache["nc"] = _build()
    nc = _cache["nc"]
    in_maps = _prepare_in_maps(x, w_attn, b_attn, w_proj)
    res = run_bass_kernel_spmd(nc, in_maps, list(range(N_CORES)), trace=trace)
    outs = []
    for b in range(B):
        full = np.empty((S, E), dtype=np.float32)
        for r_ in range(4):
            core_out = res.results[4 * b + r_]["out"]
            for qt in range(QT_N - 1):
                full[qt * 512 + r_ * 128 : qt * 512 + (r_ + 1) * 128] = core_out[
                    qt * 128 : (qt + 1) * 128
                ]
            # last query tile was reduce-scattered in two 256-token halves
            for half in range(2):
                t0 = (QT_N - 1) * 512 + half * 256
                full[t0 + r_ * 64 : t0 + (r_ + 1) * 64] = core_out[
                    (QT_N - 1) * 128 + half * 64 : (QT_N - 1) * 128 + (half + 1) * 64
                ]
        outs.append(full + b_proj[None, :])
    return np.stack(outs).astype(np.float32), res


def kernel(x, w_attn, b_attn, w_proj, b_proj):
    x = np.asarray(x, dtype=np.float32)
    w_attn = np.asarray(w_attn, dtype=np.float32)
    b_attn = np.asarray(b_attn, dtype=np.float32)
    w_proj = np.asarray(w_proj, dtype=np.float32)
    b_proj = np.asarray(b_proj, dtype=np.float32)
    out, _ = _run(x, w_attn, b_attn, w_proj, b_proj, trace=False)
    return out



# revision 8
# speedup vs baseline: 1.1455x; 1.1455x over previous
"""Trainium2 Bass kernel for causal multi-head attention block (GPT-style).

Reference computation (fp32):
    qkv = x @ w_attn + b_attn          # [B,S,3E], heads interleaved per 192 cols
    q,k,v per head (d=64), scores = q k^T / 8, causal mask, softmax
    a = softmax @ v ; h = a @ w_proj + b_proj

Sharding (8 cores): core c -> batch b = c//4, head group g = c%4 (4 heads).
Each core computes qkv for its heads, full causal attention, and a partial
c_proj over its 256 e_in rows; bf16 ReduceScatter(add) chunks per batch group
yield each core's slice of the final output. b_proj added on host.

v2 layout/schedule:
  - Phase A runs per 512-token chunk, interleaved with attention query tiles,
    so compute starts as soon as the first xT chunk lands and the first
    ReduceScatter issues ~3x earlier.
  - Scores run as fp8e4 DoubleRow matmuls (2x PE throughput): q,k are cast to
    fp8 after the bf16 projection, d split into two 32-halves on free dim.
    Head c lives on partitions 32c..32c+32 of qt8/kt8 [128, 2, S].
    The 1/sqrt(d)=1/8 scale folds into the EXP activation scale.
    K-projection bias is dropped entirely (softmax shift-invariance per query).
  - c_proj partials stream to DRAM in bf16; ReduceScatter runs in bf16 in
    halves (qt 0-2) / quarters (qt 3) for overlap and a short tail; the
    out_d DMAs issue from the gpsimd queue so a pending RS never head-of-line
    blocks the Sync DMA queue (that blocking cost the old kernel ~35us).

On-device dataflow per head pair (heads stacked on partition halves):
    QKV psum via PE (bf16, M order per pair = [A-dlo32|B-dlo32|A-dhi32|B-dhi32])
    S^T[key,q] psum = DR-fp8(kt8, qt8) per head (K=32x2)
    P = exp(S^T/8) via ACT, tri-masked on diag tiles
    a^T|denom psum[65,512] += [V_h|1]^T P  (ones col gives softmax denom)
    at = a^T * recip(denom) broadcast  -> c_proj lhsT [64, tok]
"""

import sys

import numpy as np

if "/opt/trn_rl_repo" not in sys.path:
    sys.path.insert(0, "/opt/trn_rl_repo")

B, S, E, H, D = 2, 2048, 1024, 16, 64
N_CORES = 8
PAIRS = 2  # head pairs per core
ET = 8  # e tiles of 128 over E=1024
QT_N = 4  # query tiles of 512
TT_N = 4  # token tiles of 512
VT_N = 16  # token tiles of 128 (V / c_proj)

_cache = {}


def _build():
    import concourse.mybir as mybir
    import concourse.tile as tile
    from concourse import bacc
    from contextlib import ExitStack

    f32 = mybir.dt.float32
    f8 = mybir.dt.float8e4
    bf16 = mybir.dt.bfloat16
    ALU = mybir.AluOpType
    AF = mybir.ActivationFunctionType
    DR = mybir.MatmulPerfMode.DoubleRow

    nc = bacc.Bacc(
        "TRN2", target_bir_lowering=False, debug=False, num_devices=N_CORES
    )

    xT_d = nc.declare_dram_parameter("xT", [E, S], bf16, isOutput=False)
    wqk_d = nc.declare_dram_parameter(
        "wqk", [128, PAIRS, 2, ET, 128], bf16, isOutput=False
    )
    wv_d = nc.declare_dram_parameter("wv", [128, ET, 256], bf16, isOutput=False)
    bqv_d = nc.declare_dram_parameter("bqv", [128, 2 + 256], f32, isOutput=False)
    trisel_d = nc.declare_dram_parameter("trisel", [128, 384], bf16, isOutput=False)
    wp_d = nc.declare_dram_parameter("wp", [128, PAIRS, 1024], bf16, isOutput=False)
    out_d = nc.declare_dram_parameter("out", [512, 1024], bf16, isOutput=True)

    with ExitStack() as ctx:
        ctx.enter_context(
            nc.allow_low_precision(reason="bf16/fp8 internal math, 2e-2 rel gate")
        )
        tc = ctx.enter_context(tile.TileContext(nc))
        const = ctx.enter_context(tc.tile_pool(name="const", bufs=1))
        dram = ctx.enter_context(tc.tile_pool(name="dram", bufs=1, space="DRAM"))
        psum = ctx.enter_context(tc.tile_pool(name="psum", bufs=4, space="PSUM"))
        psum_av = ctx.enter_context(tc.tile_pool(name="psum_av", bufs=2, space="PSUM"))
        pbuf = ctx.enter_context(tc.tile_pool(name="pbuf", bufs=6))

        # ---- persistent SBUF tensors ----
        xT = const.tile([128, ET, S], bf16, tag="xT")  # 4 MB
        wqk = const.tile([128, PAIRS, 2, ET, 128], bf16, tag="wqk")
        wv = const.tile([128, ET, 256], bf16, tag="wv")
        bqv = const.tile([128, 2 + 256], f32, tag="bqv")
        trisel = const.tile([128, 384], bf16, tag="trisel")
        wp = const.tile([128, PAIRS, 1024], bf16, tag="wp")
        # per-pair fp8 q/k: head hh of pair p on partitions 32hh..32hh+32 of tile p
        qt8 = [
            const.tile([64, 2, S], f8, tag=f"qt8_{p}", name=f"qt8_{p}")
            for p in range(PAIRS)
        ]
        kt8 = [
            const.tile([64, 2, S], f8, tag=f"kt8_{p}", name=f"kt8_{p}")
            for p in range(PAIRS)
        ]
        vv = const.tile([128, VT_N, 4 * 65], bf16, tag="vv")  # [key,vt,(h,d|1)]
        at = const.tile([128, PAIRS, S], bf16, tag="at")  # pair-stacked a^T

        tri = trisel[:, 0:128]
        sel = trisel.rearrange("p (a b) -> p a b", a=3)[:, 1:3, :]  # [128,2,128]

        # ---- input DMAs: weights first, then xT per 512-token chunk ----
        nc.sync.dma_start(out=wv[:], in_=wv_d[:])
        nc.sync.dma_start(out=wqk[:], in_=wqk_d[:])
        nc.sync.dma_start(out=bqv[:], in_=bqv_d[:])
        nc.sync.dma_start(out=trisel[:], in_=trisel_d[:])
        for et in range(ET):
            nc.sync.dma_start(
                out=xT[:, et, 0:512], in_=xT_d[et * 128 : (et + 1) * 128, 0:512]
            )
        nc.sync.dma_start(out=wp[:], in_=wp_d[:])
        for tt in range(1, TT_N):
            sl = slice(tt * 512, (tt + 1) * 512)
            for et in range(ET):
                nc.sync.dma_start(
                    out=xT[:, et, sl], in_=xT_d[et * 128 : (et + 1) * 128, sl]
                )
        nc.vector.memset(vv.rearrange("p t (h e) -> p t h e", h=4)[:, :, :, 64:65], 1.0)

        cc_in = []
        for qt in range(QT_N):
            cc_in.append(
                dram.tile([512, 1024], bf16, tag=f"cc_in{qt}", name=f"cc_in{qt}")
            )
        cc_outh = {}
        for qt in range(QT_N - 1):
            for half in range(2):
                cc_outh[(qt, half)] = dram.tile(
                    [64, 1024], bf16, tag=f"cc_oh{qt}_{half}", name=f"cc_oh{qt}_{half}"
                )
        cc_outq = []
        for qq in range(4):
            cc_outq.append(
                dram.tile([32, 1024], bf16, tag=f"cc_oq{qq}", name=f"cc_oq{qq}")
            )

        def phase_a(tt):
            """QKV projections for 512-token chunk tt."""
            for vt in range(4 * tt, 4 * tt + 4):
                sl = slice(vt * 128, (vt + 1) * 128)
                ps_v = psum.tile([128, 256], f32, tag="mm", name=f"psv_{vt}")
                for et in range(ET):
                    nc.tensor.matmul(
                        ps_v,
                        lhsT=xT[:, et, sl],
                        rhs=wv[:, et],
                        start=(et == 0),
                        stop=(et == ET - 1),
                    )
                nc.vector.tensor_tensor(
                    out=vv.rearrange("p t (h e) -> p t h e", h=4)[:, vt, :, 0:64],
                    in0=ps_v.rearrange("p (h e) -> p h e", h=4),
                    in1=bqv[:, 2:258].rearrange("p (h e) -> p h e", h=4),
                    op=ALU.add,
                )
            sl = slice(tt * 512, (tt + 1) * 512)
            for p in range(PAIRS):
                ps_q = psum.tile([128, 512], f32, tag="mm", name=f"psq_{p}_{tt}")
                for et in range(ET):
                    nc.tensor.matmul(
                        ps_q,
                        lhsT=wqk[:, p, 0, et],
                        rhs=xT[:, et, sl],
                        start=(et == 0),
                        stop=(et == ET - 1),
                    )
                # psum M order (j, hh, p32) -> qt8[p] partitions (hh, p32), slot j
                for j in range(2):
                    nc.vector.tensor_scalar_add(
                        qt8[p][:, j, sl],
                        ps_q[64 * j : 64 * j + 64, :],
                        bqv[64 * j : 64 * j + 64, p : p + 1],
                    )
                ps_k = psum.tile([128, 512], f32, tag="mm", name=f"psk_{p}_{tt}")
                for et in range(ET):
                    nc.tensor.matmul(
                        ps_k,
                        lhsT=wqk[:, p, 1, et],
                        rhs=xT[:, et, sl],
                        start=(et == 0),
                        stop=(et == ET - 1),
                    )
                # k bias dropped: per-query-constant shift cancels in softmax
                for j in range(2):
                    nc.scalar.activation(
                        kt8[p][:, j, sl],
                        ps_k[64 * j : 64 * j + 64, :],
                        AF.Copy,
                    )

        # c0 for the 4 diagonal key-tiles of a query tile
        diag_c0 = (0, 128, 256, 384)

        def flush(qt, den4, atu):
            """normalize (recip->broadcast->mult), c_proj, ReduceScatter for qt."""
            rec4 = pbuf.tile([128, 512], bf16, tag="recb", bufs=2, name=f"rec_{qt}")
            nc.vector.reciprocal(rec4[:], den4[:])
            for pi in range(PAIRS):
                rb = psum.tile([128, 512], f32, tag="cc", bufs=2, name=f"rb_{qt}_{pi}")
                nc.tensor.matmul(
                    rb, lhsT=sel[:, pi, :], rhs=rec4[:], start=True, stop=True
                )
                nc.vector.tensor_tensor(
                    out=at[:, pi, qt * 512 : (qt + 1) * 512],
                    in0=atu[pi][:],
                    in1=rb[:],
                    op=ALU.mult,
                )
            for tt in range(4 * qt, 4 * qt + 4):
                for nt in range(2):
                    ps_c = psum.tile([128, 512], f32, tag="cc", bufs=2)
                    for pi in range(PAIRS):
                        nc.tensor.matmul(
                            ps_c,
                            lhsT=at[:, pi, tt * 128 : (tt + 1) * 128],
                            rhs=wp[:, pi, nt * 512 : (nt + 1) * 512],
                            start=(pi == 0),
                            stop=(pi == PAIRS - 1),
                        )
                    cst = pbuf.tile(
                        [128, 512], bf16, tag="cstage", bufs=3, name=f"cst_{tt}_{nt}"
                    )
                    nc.scalar.copy(cst[:], ps_c[:])
                    nc.sync.dma_start(
                        out=cc_in[qt][
                            (tt - 4 * qt) * 128 : (tt - 4 * qt + 1) * 128,
                            nt * 512 : (nt + 1) * 512,
                        ],
                        in_=cst[:],
                    )
                j = tt - 4 * qt
                if qt < QT_N - 1 and j in (1, 3):
                    half = j // 2
                    nc.gpsimd.collective_compute(
                        "ReduceScatter",
                        mybir.AluOpType.add,
                        replica_groups=[[0, 1, 2, 3], [4, 5, 6, 7]],
                        ins=[cc_in[qt][half * 256 : (half + 1) * 256, :].opt()],
                        outs=[cc_outh[(qt, half)][:].opt()],
                    )
                    nc.gpsimd.dma_start(
                        out=out_d[qt * 128 + half * 64 : qt * 128 + (half + 1) * 64, :],
                        in_=cc_outh[(qt, half)][:],
                    )
                elif qt == QT_N - 1:
                    nc.gpsimd.collective_compute(
                        "ReduceScatter",
                        mybir.AluOpType.add,
                        replica_groups=[[0, 1, 2, 3], [4, 5, 6, 7]],
                        ins=[cc_in[qt][j * 128 : (j + 1) * 128, :].opt()],
                        outs=[cc_outq[j][:].opt()],
                    )
                    nc.gpsimd.dma_start(
                        out=out_d[384 + j * 32 : 384 + (j + 1) * 32, :],
                        in_=cc_outq[j][:],
                    )

        def attention(qt):
            """scores -> exp -> AV for query tile qt; returns (den4, atu_pair)."""
            den4 = pbuf.tile([128, 512], f32, tag="den", bufs=2, name=f"den_{qt}")
            nc.vector.memset(den4[:], 1.0)
            atu_pair = []
            for p in range(PAIRS):
                av = []
                for hh in range(2):
                    av.append(
                        psum_av.tile([65, 512], f32, tag="av", name=f"av_{p}_{qt}_{hh}")
                    )
                nkt = 4 * qt + 4
                for kt in range(nkt):
                    j = kt - 4 * qt
                    c0 = diag_c0[j] if j >= 0 else 0
                    n = 512 - c0
                    ps_s = []
                    pts = []
                    for hh in range(2):
                        base = 32 * hh
                        ps_s.append(
                            psum.tile(
                                [128, 512], f32, tag="mm",
                                name=f"ps_s_{p}_{qt}_{kt}_{hh}",
                            )
                        )
                        nc.tensor.matmul(
                            ps_s[hh][:, 0:n],
                            lhsT=kt8[p][
                                base : base + 32, :, kt * 128 : (kt + 1) * 128
                            ],
                            rhs=qt8[p][
                                base : base + 32, :, qt * 512 + c0 : (qt + 1) * 512
                            ],
                            start=True,
                            stop=True,
                            perf_mode=DR,
                        )
                    for hh in range(2):
                        pt = pbuf.tile(
                            [128, 512], bf16, tag="p", bufs=8,
                            name=f"pt_{p}_{qt}_{kt}_{hh}",
                        )
                        pts.append(pt)
                        nc.scalar.activation(
                            pt[:, 0:n], ps_s[hh][:, 0:n], AF.Exp, scale=0.125
                        )
                        if j >= 0:
                            nc.vector.tensor_tensor(
                                out=pt[:, 0:128], in0=pt[:, 0:128], in1=tri,
                                op=ALU.mult,
                            )
                    for hh in range(2):
                        h_idx = 2 * p + hh
                        nc.tensor.matmul(
                            av[hh][:, c0:512],
                            lhsT=vv[:, kt, h_idx * 65 : (h_idx + 1) * 65],
                            rhs=pts[hh][:, 0:n],
                            start=(kt == 0),
                            stop=(kt == nkt - 1),
                        )
                for hh in range(2):
                    h_idx = 2 * p + hh
                    nc.vector.tensor_copy(
                        out=den4[h_idx * 32 : h_idx * 32 + 1, :], in_=av[hh][64:65, :]
                    )
                atu2 = pbuf.tile(
                    [128, 512], f32, tag="atu", bufs=4, name=f"atu_{p}_{qt}"
                )
                nc.vector.tensor_copy(out=atu2[0:64, :], in_=av[0][0:64, :])
                nc.vector.tensor_copy(out=atu2[64:128, :], in_=av[1][0:64, :])
                atu_pair.append(atu2)
            return den4, atu_pair

        pending = None
        for tt in range(TT_N):
            phase_a(tt)
            if pending is not None:
                flush(*pending)
            den4, atu_pair = attention(tt)
            pending = (tt, den4, atu_pair)
        flush(*pending)

    nc.compile()
    return nc


def _prepare_in_maps(x, w_attn, b_attn, w_proj):
    import ml_dtypes

    bf = ml_dtypes.bfloat16
    in_maps = []
    trisel = np.zeros((128, 384), dtype=bf)
    trisel[:, 0:128] = np.triu(np.ones((128, 128), dtype=bf))
    for core in range(N_CORES):
        b, g = core // 4, core % 4
        heads = [4 * g + i for i in range(4)]
        xT = np.ascontiguousarray(x[b].T)  # [1024, 2048]
        wqk_blocks = []
        bq_cols = []
        for pr in range(PAIRS):
            hA, hB = heads[2 * pr], heads[2 * pr + 1]
            qk_pair = []
            for off in (0, 64):  # q cols, k cols
                blk = np.concatenate(
                    [
                        w_attn[:, hA * 192 + off : hA * 192 + off + 64],
                        w_attn[:, hB * 192 + off : hB * 192 + off + 64],
                    ],
                    axis=1,
                )  # [1024, 128] cols (hh, d)
                # reorder cols to (j, hh, p32): col = hh*64 + j*32 + p
                blk = (
                    blk.reshape(1024, 2, 2, 32)
                    .transpose(0, 2, 1, 3)
                    .reshape(1024, 128)
                )
                # [1024,128] -> [128part, 8et, 128]
                qk_pair.append(blk.reshape(ET, 128, 128).transpose(1, 0, 2))
            wqk_blocks.append(np.stack(qk_pair, axis=1))  # [128, 2, 8, 128]
            bqA = b_attn[hA * 192 : hA * 192 + 64]
            bqB = b_attn[hB * 192 : hB * 192 + 64]
            bq_cols.append(
                np.concatenate([bqA[0:32], bqB[0:32], bqA[32:64], bqB[32:64]])
            )
        wqk_h = np.stack(wqk_blocks, axis=1)  # [128, 2pair, 2qk, 8, 128]
        wv_blk = np.concatenate(
            [w_attn[:, h * 192 + 128 : h * 192 + 192] for h in heads], axis=1
        )  # [1024, 256]
        wv_h = wv_blk.reshape(ET, 128, 256).transpose(1, 0, 2)  # [128, 8, 256]
        bv_row = np.concatenate(
            [b_attn[h * 192 + 128 : h * 192 + 192] for h in heads]
        )  # [256]
        bqv = np.zeros((128, 258), dtype=np.float32)
        bqv[:, 0] = bq_cols[0]
        bqv[:, 1] = bq_cols[1]
        bqv[:, 2:258] = np.broadcast_to(bv_row, (128, 256))
        wp_h = np.empty((128, PAIRS, 1024), dtype=np.float32)
        ts = trisel.copy()
        for pr in range(PAIRS):
            hA, hB = heads[2 * pr], heads[2 * pr + 1]
            wp_h[0:64, pr, :] = w_proj[hA * 64 : (hA + 1) * 64, :]
            wp_h[64:128, pr, :] = w_proj[hB * 64 : (hB + 1) * 64, :]
            ts[(2 * pr) * 32, 128 + pr * 128 : 128 + pr * 128 + 64] = 1.0
            ts[(2 * pr + 1) * 32, 128 + pr * 128 + 64 : 128 + pr * 128 + 128] = 1.0
        in_maps.append(
            {
                "xT": np.ascontiguousarray(xT.astype(bf)),
                "wqk": np.ascontiguousarray(wqk_h.astype(bf)),
                "wv": np.ascontiguousarray(wv_h.astype(bf)),
                "bqv": bqv,
                "trisel": np.ascontiguousarray(ts),
                "wp": np.ascontiguousarray(wp_h.astype(bf)),
            }
        )
    return in_maps


def _run(x, w_attn, b_attn, w_proj, b_proj, trace=False):
    from concourse.bass_utils import run_bass_kernel_spmd

    if "nc" not in _cache:
        _cache["nc"] = _build()
    nc = _cache["nc"]
    in_maps = _prepare_in_maps(x, w_attn, b_attn, w_proj)
    res = run_bass_kernel_spmd(nc, in_maps, list(range(N_CORES)), trace=trace)
    outs = []
    for b in range(B):
        full = np.empty((S, E), dtype=np.float32)
        for r_ in range(4):
            core_out = np.asarray(res.results[4 * b + r_]["out"], dtype=np.float32)
            for qt in range(QT_N - 1):
                for half in range(2):
                    t0 = qt * 512 + half * 256 + r_ * 64
                    full[t0 : t0 + 64] = core_out[
                        qt * 128 + half * 64 : qt * 128 + (half + 1) * 64
                    ]
            for qq in range(4):
                t0 = 1536 + qq * 128 + r_ * 32
                full[t0 : t0 + 32] = core_out[384 + qq * 32 : 384 + (qq + 1) * 32]
        outs.append(full + b_proj[None, :])
    return np.stack(outs).astype(np.float32), res


def kernel(x, w_attn, b_attn, w_proj, b_proj):
    x = np.asarray(x, dtype=np.float32)
    w_attn = np.asarray(w_attn, dtype=np.float32)
    b_attn = np.asarray(b_attn, dtype=np.float32)
    w_proj = np.asarray(w_proj, dtype=np.float32)
    b_proj = np.asarray(b_proj, dtype=np.float32)
    out, _ = _run(x, w_attn, b_attn, w_proj, b_proj, trace=False)
    return out


# revision 21
# speedup vs baseline: 1.2031x; 1.0503x over previous
"""Trainium2 Bass kernel for causal multi-head attention block (GPT-style).

Reference computation (fp32):
    qkv = x @ w_attn + b_attn          # [B,S,3E], heads interleaved per 192 cols
    q,k,v per head (d=64), scores = q k^T / 8, causal mask, softmax
    a = softmax @ v ; h = a @ w_proj + b_proj

Sharding (8 cores): core c -> batch b = c//4, head group g = c%4 (4 heads).
Each core computes qkv for its heads, full causal attention, and a partial
c_proj over its 256 e_in rows; bf16 ReduceScatter(add) chunks per batch group
yield each core's slice of the final output. b_proj added on host.

v3 layout/schedule:
  - Phase A runs per 512-token chunk, interleaved with attention query tiles,
    so compute starts as soon as the first xT chunk lands and the first
    ReduceScatter issues ~3x earlier.
  - Scores are bf16 K=64 matmuls on the two 64-row PE groups (h0/h64), which
    the PE executes concurrently -- two heads' score streams overlap.
    The 1/sqrt(d)=1/8 scale folds into the EXP activation scale.
    K-projection bias is dropped entirely (softmax shift-invariance per query).
  - c_proj partials stream to DRAM in bf16; ReduceScatter runs in bf16 in
    halves for overlap and a short tail; the out_d DMAs issue from the gpsimd
    queue so a pending RS never head-of-line blocks the Sync DMA queue (that
    blocking cost the old kernel ~35us).

On-device dataflow per head pair (heads stacked on partition halves):
    QKV psum via PE (bf16, M order per pair = [A-d64 | B-d64])
    S^T[key,q] psum = KT_h^T QT_h (K=64; heads on row groups h0/h64)
    P = exp(S^T/8) via ACT, tri-masked on diag tiles
    a^T|denom psum[65,512] += [V_h|1]^T P  (ones col gives softmax denom)
    at = a^T * recip(denom) broadcast  -> c_proj lhsT [64, tok]
"""

import sys

import numpy as np

if "/opt/trn_rl_repo" not in sys.path:
    sys.path.insert(0, "/opt/trn_rl_repo")

B, S, E, H, D = 2, 2048, 1024, 16, 64
N_CORES = 8
PAIRS = 2  # head pairs per core
ET = 8  # e tiles of 128 over E=1024
QT_N = 4  # query tiles of 512
TT_N = 4  # token tiles of 512
VT_N = 16  # token tiles of 128 (V / c_proj)

_cache = {}


def _build():
    import concourse.mybir as mybir
    import concourse.tile as tile
    from concourse import bacc
    from contextlib import ExitStack

    f32 = mybir.dt.float32
    bf16 = mybir.dt.bfloat16
    ALU = mybir.AluOpType
    AF = mybir.ActivationFunctionType

    nc = bacc.Bacc(
        "TRN2", target_bir_lowering=False, debug=False, num_devices=N_CORES
    )

    xT_d = nc.declare_dram_parameter("xT", [E, S], bf16, isOutput=False)
    wqk_d = nc.declare_dram_parameter(
        "wqk", [128, PAIRS, 2, ET, 128], bf16, isOutput=False
    )
    wv_d = nc.declare_dram_parameter("wv", [128, ET, 256], bf16, isOutput=False)
    bqv_d = nc.declare_dram_parameter("bqv", [128, 2 + 256], f32, isOutput=False)
    trisel_d = nc.declare_dram_parameter("trisel", [128, 384], bf16, isOutput=False)
    wp_d = nc.declare_dram_parameter("wp", [128, PAIRS, 1024], bf16, isOutput=False)
    out_d = nc.declare_dram_parameter("out", [512, 1024], bf16, isOutput=True)

    with ExitStack() as ctx:
        ctx.enter_context(
            nc.allow_low_precision(reason="bf16/fp8 internal math, 2e-2 rel gate")
        )
        tc = ctx.enter_context(tile.TileContext(nc))
        const = ctx.enter_context(tc.tile_pool(name="const", bufs=1))
        dram = ctx.enter_context(tc.tile_pool(name="dram", bufs=1, space="DRAM"))
        psum = ctx.enter_context(tc.tile_pool(name="psum", bufs=4, space="PSUM"))
        psum_av = ctx.enter_context(tc.tile_pool(name="psum_av", bufs=2, space="PSUM"))
        pbuf = ctx.enter_context(tc.tile_pool(name="pbuf", bufs=6))

        # ---- persistent SBUF tensors ----
        xT = const.tile([128, ET, S], bf16, tag="xT")  # 4 MB
        wqk = const.tile([128, PAIRS, 2, ET, 128], bf16, tag="wqk")
        wv = const.tile([128, ET, 256], bf16, tag="wv")
        bqv = const.tile([128, 2 + 256], f32, tag="bqv")
        trisel = const.tile([128, 384], bf16, tag="trisel")
        wp = const.tile([128, PAIRS, 1024], bf16, tag="wp")
        qt_sb = const.tile([128, PAIRS, S], bf16, tag="qt")  # rows 0-63 head A
        kt_sb = const.tile([128, PAIRS, S], bf16, tag="kt")
        vv = const.tile([128, VT_N, 4 * 65], bf16, tag="vv")  # [key,vt,(h,d|1)]
        at = const.tile([128, PAIRS, S], bf16, tag="at")  # pair-stacked a^T

        tri = trisel[:, 0:128]
        sel = trisel.rearrange("p (a b) -> p a b", a=3)[:, 1:3, :]  # [128,2,128]

        # ---- input DMAs: first V-proj needs only xT[:,0,chunk0] + wv ----
        nc.sync.dma_start(out=xT[:, 0, 0:512], in_=xT_d[0:128, 0:512])
        nc.sync.dma_start(out=wv[:], in_=wv_d[:])
        for et in range(1, ET):
            nc.sync.dma_start(
                out=xT[:, et, 0:512], in_=xT_d[et * 128 : (et + 1) * 128, 0:512]
            )
        nc.sync.dma_start(out=bqv[:], in_=bqv_d[:])
        nc.sync.dma_start(out=wqk[:], in_=wqk_d[:])
        nc.sync.dma_start(out=trisel[:], in_=trisel_d[:])
        nc.sync.dma_start(out=wp[:], in_=wp_d[:])
        for tt in range(1, TT_N):
            sl = slice(tt * 512, (tt + 1) * 512)
            for et in range(ET):
                nc.sync.dma_start(
                    out=xT[:, et, sl], in_=xT_d[et * 128 : (et + 1) * 128, sl]
                )
        nc.vector.memset(vv.rearrange("p t (h e) -> p t h e", h=4)[:, :, :, 64:65], 1.0)

        cc_in = []
        for qt in range(QT_N):
            cc_in.append(
                dram.tile([512, 1024], bf16, tag=f"cc_in{qt}", name=f"cc_in{qt}")
            )
        cc_outh = {}
        for qt in range(QT_N):
            for half in range(2):
                cc_outh[(qt, half)] = dram.tile(
                    [64, 1024], bf16, tag=f"cc_oh{qt}_{half}", name=f"cc_oh{qt}_{half}"
                )

        def phase_a(tt):
            """QKV projections for 512-token chunk tt."""
            for vt in range(4 * tt, 4 * tt + 4):
                sl = slice(vt * 128, (vt + 1) * 128)
                ps_v = psum.tile([128, 256], f32, tag="mm", name=f"psv_{vt}")
                for et in range(ET):
                    nc.tensor.matmul(
                        ps_v,
                        lhsT=xT[:, et, sl],
                        rhs=wv[:, et],
                        start=(et == 0),
                        stop=(et == ET - 1),
                    )
                nc.vector.tensor_tensor(
                    out=vv.rearrange("p t (h e) -> p t h e", h=4)[:, vt, :, 0:64],
                    in0=ps_v.rearrange("p (h e) -> p h e", h=4),
                    in1=bqv[:, 2:258].rearrange("p (h e) -> p h e", h=4),
                    op=ALU.add,
                )
            sl = slice(tt * 512, (tt + 1) * 512)
            for p in range(PAIRS):
                ps_q = psum.tile([128, 512], f32, tag="mm", name=f"psq_{p}_{tt}")
                for et in range(ET):
                    nc.tensor.matmul(
                        ps_q,
                        lhsT=wqk[:, p, 0, et],
                        rhs=xT[:, et, sl],
                        start=(et == 0),
                        stop=(et == ET - 1),
                    )
                nc.vector.tensor_scalar_add(
                    qt_sb[:, p, sl], ps_q, bqv[:, p : p + 1]
                )
                ps_k = psum.tile([128, 512], f32, tag="mm", name=f"psk_{p}_{tt}")
                for et in range(ET):
                    nc.tensor.matmul(
                        ps_k,
                        lhsT=wqk[:, p, 1, et],
                        rhs=xT[:, et, sl],
                        start=(et == 0),
                        stop=(et == ET - 1),
                    )
                # k bias dropped: per-query-constant shift cancels in softmax
                nc.scalar.copy(kt_sb[:, p, sl], ps_k[:])

        # c0 for the 4 diagonal key-tiles of a query tile
        diag_c0 = (0, 128, 256, 384)

        def flush(qt, den4, atu):
            """normalize (recip->broadcast->mult), c_proj, ReduceScatter for qt."""
            rec4 = pbuf.tile([128, 512], bf16, tag="recb", bufs=2, name=f"rec_{qt}")
            nc.vector.reciprocal(rec4[:], den4[:])
            for pi in range(PAIRS):
                rb = psum.tile([128, 512], f32, tag="cc", bufs=2, name=f"rb_{qt}_{pi}")
                nc.tensor.matmul(
                    rb, lhsT=sel[:, pi, :], rhs=rec4[:], start=True, stop=True
                )
                nc.vector.tensor_tensor(
                    out=at[:, pi, qt * 512 : (qt + 1) * 512],
                    in0=atu[pi][:],
                    in1=rb[:],
                    op=ALU.mult,
                )
            for tt in range(4 * qt, 4 * qt + 4):
                for nt in range(2):
                    ps_c = psum.tile([128, 512], f32, tag="cc", bufs=2)
                    for pi in range(PAIRS):
                        nc.tensor.matmul(
                            ps_c,
                            lhsT=at[:, pi, tt * 128 : (tt + 1) * 128],
                            rhs=wp[:, pi, nt * 512 : (nt + 1) * 512],
                            start=(pi == 0),
                            stop=(pi == PAIRS - 1),
                        )
                    cst = pbuf.tile(
                        [128, 512], bf16, tag="cstage", bufs=3, name=f"cst_{tt}_{nt}"
                    )
                    nc.vector.tensor_copy(out=cst[:], in_=ps_c[:])
                    nc.sync.dma_start(
                        out=cc_in[qt][
                            (tt - 4 * qt) * 128 : (tt - 4 * qt + 1) * 128,
                            nt * 512 : (nt + 1) * 512,
                        ],
                        in_=cst[:],
                    )
                j = tt - 4 * qt
                if j in (1, 3):
                    half = j // 2
                    nc.gpsimd.collective_compute(
                        "ReduceScatter",
                        mybir.AluOpType.add,
                        replica_groups=[[0, 1, 2, 3], [4, 5, 6, 7]],
                        ins=[cc_in[qt][half * 256 : (half + 1) * 256, :].opt()],
                        outs=[cc_outh[(qt, half)][:].opt()],
                    )
                    nc.gpsimd.dma_start(
                        out=out_d[qt * 128 + half * 64 : qt * 128 + (half + 1) * 64, :],
                        in_=cc_outh[(qt, half)][:],
                    )

        def attention(qt):
            """scores -> exp -> AV for query tile qt; returns (den4, atu_pair)."""
            den4 = pbuf.tile([128, 512], f32, tag="den", bufs=2, name=f"den_{qt}")
            nc.vector.memset(den4[:], 1.0)
            atu_pair = []
            for p in range(PAIRS):
                av = []
                for hh in range(2):
                    av.append(
                        psum_av.tile([65, 512], f32, tag="av", name=f"av_{p}_{qt}_{hh}")
                    )
                nkt = 4 * qt + 4
                for kt in range(nkt):
                    j = kt - 4 * qt
                    c0 = diag_c0[j] if j >= 0 else 0
                    n = 512 - c0
                    ps_s = []
                    pts = []
                    for hh in range(2):
                        base = 64 * hh
                        ps_s.append(
                            psum.tile(
                                [128, 512], f32, tag="mm",
                                name=f"ps_s_{p}_{qt}_{kt}_{hh}",
                            )
                        )
                        nc.tensor.matmul(
                            ps_s[hh][:, 0:n],
                            lhsT=kt_sb[
                                base : base + 64, p, kt * 128 : (kt + 1) * 128
                            ],
                            rhs=qt_sb[
                                base : base + 64, p, qt * 512 + c0 : (qt + 1) * 512
                            ],
                            start=True,
                            stop=True,
                        )
                    for hh in range(2):
                        pt = pbuf.tile(
                            [128, 512], bf16, tag="p", bufs=8,
                            name=f"pt_{p}_{qt}_{kt}_{hh}",
                        )
                        pts.append(pt)
                        nc.scalar.activation(
                            pt[:, 0:n], ps_s[hh][:, 0:n], AF.Exp, scale=0.125
                        )
                        if j >= 0:
                            nc.vector.tensor_tensor(
                                out=pt[:, 0:128], in0=pt[:, 0:128], in1=tri,
                                op=ALU.mult,
                            )
                    for hh in range(2):
                        h_idx = 2 * p + hh
                        nc.tensor.matmul(
                            av[hh][:, c0:512],
                            lhsT=vv[:, kt, h_idx * 65 : (h_idx + 1) * 65],
                            rhs=pts[hh][:, 0:n],
                            start=(kt == 0),
                            stop=(kt == nkt - 1),
                        )
                for hh in range(2):
                    h_idx = 2 * p + hh
                    nc.vector.tensor_copy(
                        out=den4[h_idx * 32 : h_idx * 32 + 1, :], in_=av[hh][64:65, :]
                    )
                atu2 = pbuf.tile(
                    [128, 512], f32, tag="atu", bufs=4, name=f"atu_{p}_{qt}"
                )
                nc.vector.tensor_copy(out=atu2[0:64, :], in_=av[0][0:64, :])
                nc.vector.tensor_copy(out=atu2[64:128, :], in_=av[1][0:64, :])
                atu_pair.append(atu2)
            return den4, atu_pair

        pending = None
        for tt in range(TT_N):
            phase_a(tt)
            if pending is not None:
                flush(*pending)
            den4, atu_pair = attention(tt)
            pending = (tt, den4, atu_pair)
        flush(*pending)

    nc.compile()
    return nc


def _prepare_in_maps(x, w_attn, b_attn, w_proj):
    import ml_dtypes

    bf = ml_dtypes.bfloat16
    in_maps = []
    trisel = np.zeros((128, 384), dtype=bf)
    trisel[:, 0:128] = np.triu(np.ones((128, 128), dtype=bf))
    for core in range(N_CORES):
        b, g = core // 4, core % 4
        heads = [4 * g + i for i in range(4)]
        xT = np.ascontiguousarray(x[b].T)  # [1024, 2048]
        wqk_blocks = []
        bq_cols = []
        for pr in range(PAIRS):
            hA, hB = heads[2 * pr], heads[2 * pr + 1]
            qk_pair = []
            for off in (0, 64):  # q cols, k cols
                blk = np.concatenate(
                    [
                        w_attn[:, hA * 192 + off : hA * 192 + off + 64],
                        w_attn[:, hB * 192 + off : hB * 192 + off + 64],
                    ],
                    axis=1,
                )  # [1024, 128] cols (hh, d)
                # [1024,128] -> [128part, 8et, 128]
                qk_pair.append(blk.reshape(ET, 128, 128).transpose(1, 0, 2))
            wqk_blocks.append(np.stack(qk_pair, axis=1))  # [128, 2, 8, 128]
            bq_cols.append(
                np.concatenate(
                    [b_attn[hA * 192 : hA * 192 + 64], b_attn[hB * 192 : hB * 192 + 64]]
                )
            )
        wqk_h = np.stack(wqk_blocks, axis=1)  # [128, 2pair, 2qk, 8, 128]
        wv_blk = np.concatenate(
            [w_attn[:, h * 192 + 128 : h * 192 + 192] for h in heads], axis=1
        )  # [1024, 256]
        wv_h = wv_blk.reshape(ET, 128, 256).transpose(1, 0, 2)  # [128, 8, 256]
        bv_row = np.concatenate(
            [b_attn[h * 192 + 128 : h * 192 + 192] for h in heads]
        )  # [256]
        bqv = np.zeros((128, 258), dtype=np.float32)
        bqv[:, 0] = bq_cols[0]
        bqv[:, 1] = bq_cols[1]
        bqv[:, 2:258] = np.broadcast_to(bv_row, (128, 256))
        wp_h = np.empty((128, PAIRS, 1024), dtype=np.float32)
        ts = trisel.copy()
        for pr in range(PAIRS):
            hA, hB = heads[2 * pr], heads[2 * pr + 1]
            wp_h[0:64, pr, :] = w_proj[hA * 64 : (hA + 1) * 64, :]
            wp_h[64:128, pr, :] = w_proj[hB * 64 : (hB + 1) * 64, :]
            ts[(2 * pr) * 32, 128 + pr * 128 : 128 + pr * 128 + 64] = 1.0
            ts[(2 * pr + 1) * 32, 128 + pr * 128 + 64 : 128 + pr * 128 + 128] = 1.0
        in_maps.append(
            {
                "xT": np.ascontiguousarray(xT.astype(bf)),
                "wqk": np.ascontiguousarray(wqk_h.astype(bf)),
                "wv": np.ascontiguousarray(wv_h.astype(bf)),
                "bqv": bqv,
                "trisel": np.ascontiguousarray(ts),
                "wp": np.ascontiguousarray(wp_h.astype(bf)),
            }
        )
    return in_maps


def _run(x, w_attn, b_attn, w_proj, b_proj, trace=False):
    from concourse.bass_utils import run_bass_kernel_spmd

    if "nc" not in _cache:
        _cache["nc"] = _build()
    nc = _cache["nc"]
    in_maps = _prepare_in_maps(x, w_attn, b_attn, w_proj)
    res = run_bass_kernel_spmd(nc, in_maps, list(range(N_CORES)), trace=trace)
    outs = []
    for b in range(B):
        full = np.empty((S, E), dtype=np.float32)
        for r_ in range(4):
            core_out = np.asarray(res.results[4 * b + r_]["out"], dtype=np.float32)
            for qt in range(QT_N):
                for half in range(2):
                    t0 = qt * 512 + half * 256 + r_ * 64
                    full[t0 : t0 + 64] = core_out[
                        qt * 128 + half * 64 : qt * 128 + (half + 1) * 64
                    ]
        outs.append(full + b_proj[None, :])
    return np.stack(outs).astype(np.float32), res


def kernel(x, w_attn, b_attn, w_proj, b_proj):
    x = np.asarray(x, dtype=np.float32)
    w_attn = np.asarray(w_attn, dtype=np.float32)
    b_attn = np.asarray(b_attn, dtype=np.float32)
    w_proj = np.asarray(w_proj, dtype=np.float32)
    b_proj = np.asarray(b_proj, dtype=np.float32)
    out, _ = _run(x, w_attn, b_attn, w_proj, b_proj, trace=False)
    return out
